# revision 35
# baseline (speedup 1.0000x reference)
"""Trainium2 Bass kernel for AffinityNeuralNetworkCliffNet (gnn_message_passing).

Strategy: data-parallel over graphs (512 graphs/core on 8 cores). Host does
layout prep only (shard / pad / transpose / one-hot segment matrices / bf16
cast); all FLOPs over node data run on-device.

Per-core layout (identical program on all cores, data differs):
  - comp nodes grouped into 4 groups of 128 graphs, each group padded to 8704
    node slots (68 chunks of 128) -> NC_PAD = 34816.
  - prot nodes slotted: 112 slots per graph (dummy slots get a feature vector v
    with p_aff_W^T v + p_aff_b = -1e4, so emb ~ -1000 = -inf for seg-max).
  - Segment sums / gathers are one-hot matmuls (Gn: [node,128-graph-group],
    Gg: [128-graph-group, node]); seg-max is a contiguous-slot max tournament.
"""

import sys
import numpy as np

sys.path.insert(0, "/opt/trn_rl_repo")

import ml_dtypes  # noqa: E402

BF16 = ml_dtypes.bfloat16

B = 4096
H = 256
N = 262144
NCORES = 8
GPC = 512              # graphs per core
NGROUP = 4             # graph groups per core (128 graphs each)
GRP_G = 128            # graphs per group
GRP_SLOTS = 9216       # comp node slots per group (72 chunks of 128)
NC_PAD = NGROUP * GRP_SLOTS          # 36864 comp slots/core
NBIG = NC_PAD // 512                 # 72 big chunks of 512
BIG_PER_GROUP = GRP_SLOTS // 512     # 18
STAGE_BIGS = 3         # big-chunks per DMA stage
STAGES = BIG_PER_GROUP // STAGE_BIGS   # 6
PSLOT = 112            # prot slots per graph
NP_PAD = GPC * PSLOT   # 57344 prot slots/core
P_SUPER = 7168         # prot super-chunk cols (64 graphs)
NSUPER = NP_PAD // P_SUPER           # 8
P_SUB = 448            # prot matmul sub-chunk (4 graphs)

USE_LRELU = True       # ScalarE Lrelu(alpha=0.1); fallback: DVE 2-op leaky


# ----------------------------------------------------------------------------
# Host-side prep
# ----------------------------------------------------------------------------

def _prep_core(core, comp_feature, prot_feature, batch_comp, batch_prot, v_dummy):
    """Build one core's input arrays. Returns (in_map_part, meta)."""
    g0 = core * GPC

    # ---- comp side: group-padded layout --------------------------------
    compT = np.zeros((H, NC_PAD), dtype=BF16)
    Gn = np.zeros((NC_PAD, GRP_G), dtype=BF16)
    Gg = np.zeros((NBIG, GRP_G, 512), dtype=BF16)
    # map: padded slot -> original node index (or -1)
    slot2node = np.full(NC_PAD, -1, dtype=np.int64)

    for grp in range(NGROUP):
        glo = g0 + grp * GRP_G
        ghi = glo + GRP_G
        lo = np.searchsorted(batch_comp, glo, side="left")
        hi = np.searchsorted(batch_comp, ghi, side="left")
        n = hi - lo
        assert n <= GRP_SLOTS, f"comp group overflow: {n} > {GRP_SLOTS}"
        base = grp * GRP_SLOTS
        compT[:, base:base + n] = comp_feature[lo:hi].T.astype(BF16)
        slot2node[base:base + n] = np.arange(lo, hi)
        lids = batch_comp[lo:hi] - glo          # 0..127 within group
        Gn[base + np.arange(n), lids] = 1.0
        # Gg planes for this group's big chunks
        for bc in range(grp * BIG_PER_GROUP, (grp + 1) * BIG_PER_GROUP):
            s = bc * 512
            js = np.arange(512)
            sl = slot2node[s:s + 512]
            real = sl >= 0
            if real.any():
                lid = batch_comp[sl[real]] - glo
                Gg[bc, lid, js[real]] = 1.0

    # ---- prot side: 112-slot layout ------------------------------------
    protT = np.empty((H, NP_PAD), dtype=BF16)
    protT[:] = v_dummy[:, None].astype(BF16)
    for g in range(GPC):
        lo = np.searchsorted(batch_prot, g0 + g, side="left")
        hi = np.searchsorted(batch_prot, g0 + g + 1, side="left")
        n = hi - lo
        assert n <= PSLOT, f"prot graph overflow: {n} > {PSLOT}"
        protT[:, g * PSLOT:g * PSLOT + n] = prot_feature[lo:hi].T.astype(BF16)

    part = {
        "compT": compT,
        "protT": protT,
        "Gn": Gn,
        "Gg": Gg,
    }
    return part, slot2node


def _weights_maps(inp):
    """Shared (replicated) weight tensors."""
    f32 = np.float32
    def bf(x):
        return np.ascontiguousarray(x, dtype=np.float32).astype(BF16)

    w = {}
    w["w_caff"] = bf(inp["c_aff_W"])                      # [256,256]
    w["w_csup"] = bf(inp["c_sup_W"])
    w["w_paff"] = bf(inp["p_aff_W"])
    w["w_aW1a"] = bf(inp["alpha_W1"][:H])                 # lhsT for TL h_alpha
    w["w_rW1a"] = bf(inp["raw_W1"][:H])                   # rhs for NM h_raw
    w["w_W1b"] = bf(np.concatenate([inp["raw_W1"][H:2 * H],
                                    inp["alpha_W1"][H:2 * H]], axis=1))  # [256,512]
    w["w_W1c"] = bf(np.concatenate([inp["raw_W1"][2 * H:],
                                    inp["alpha_W1"][2 * H:]], axis=1))   # [256,512]
    w["b1_row"] = bf(np.concatenate([inp["raw_b1"], inp["alpha_b1"]])[None, :])  # [1,512]
    w["w_aW2"] = bf(inp["alpha_W2"])                      # [256,1]
    w["b_aW2"] = bf(np.asarray(inp["alpha_b2"]).reshape(1, 1))
    w["b_csup_row"] = bf(np.asarray(inp["c_sup_b"])[None, :])   # [1,256]
    w["b_caff"] = np.ascontiguousarray(np.asarray(inp["c_aff_b"], f32).reshape(H, 1))
    w["b_paff"] = np.ascontiguousarray(np.asarray(inp["p_aff_b"], f32).reshape(H, 1))
    w["w_rW2"] = np.ascontiguousarray(inp["raw_W2"], dtype=f32)       # [256,256]
    w["rawb2_row"] = np.ascontiguousarray(np.asarray(inp["raw_b2"], f32)[None, :])
    w["w_oW1"] = np.ascontiguousarray(inp["out_W1"], dtype=f32)
    w["outb1_row"] = np.ascontiguousarray(np.asarray(inp["out_b1"], f32)[None, :])
    w["w_oW2"] = np.ascontiguousarray(inp["out_W2"], dtype=f32)       # [256,1]
    w["outb2_row"] = np.ascontiguousarray(np.asarray(inp["out_b2"], f32).reshape(1, 1))
    w["id128"] = np.eye(128, dtype=f32)
    return w


# ----------------------------------------------------------------------------
# Device program (identical for all cores)
# ----------------------------------------------------------------------------

def _build_program():
    import concourse.bass as bass  # noqa: F401
    import concourse.bacc as bacc
    import concourse.mybir as mybir
    import concourse.tile as tile

    dt = mybir.dt
    AF = mybir.ActivationFunctionType
    OP = mybir.AluOpType

    nc = bacc.Bacc("TRN2", target_bir_lowering=False)

    # ---- dram tensors ---------------------------------------------------
    d_compT = nc.dram_tensor("compT", [H, NC_PAD], dt.bfloat16, kind="ExternalInput")
    d_protT = nc.dram_tensor("protT", [H, NP_PAD], dt.bfloat16, kind="ExternalInput")
    d_Gn = nc.dram_tensor("Gn", [NC_PAD, GRP_G], dt.bfloat16, kind="ExternalInput")
    d_Gg = nc.dram_tensor("Gg", [NBIG, GRP_G, 512], dt.bfloat16, kind="ExternalInput")

    d_w = {}
    for name, shape, ddt in [
        ("w_caff", [H, H], dt.bfloat16), ("w_csup", [H, H], dt.bfloat16),
        ("w_paff", [H, H], dt.bfloat16), ("w_aW1a", [H, H], dt.bfloat16),
        ("w_rW1a", [H, H], dt.bfloat16), ("w_W1b", [H, 2 * H], dt.bfloat16),
        ("w_W1c", [H, 2 * H], dt.bfloat16), ("b1_row", [1, 2 * H], dt.bfloat16),
        ("w_aW2", [H, 1], dt.bfloat16), ("b_aW2", [1, 1], dt.bfloat16),
        ("b_csup_row", [1, H], dt.bfloat16),
        ("b_caff", [H, 1], dt.float32), ("b_paff", [H, 1], dt.float32),
        ("w_rW2", [H, H], dt.float32), ("rawb2_row", [1, H], dt.float32),
        ("w_oW1", [H, H], dt.float32), ("outb1_row", [1, H], dt.float32),
        ("w_oW2", [H, 1], dt.float32), ("outb2_row", [1, 1], dt.float32),
        ("id128", [128, 128], dt.float32),
    ]:
        d_w[name] = nc.dram_tensor(name, shape, ddt, kind="ExternalInput")

    d_alpha = nc.dram_tensor("out_alpha", [128, NC_PAD // 128], dt.float32,
                             kind="ExternalOutput")
    d_vec = nc.dram_tensor("out_vector", [GPC, H], dt.float32, kind="ExternalOutput")
    d_aff = nc.dram_tensor("out_aff", [GPC, 1], dt.float32, kind="ExternalOutput")

    NCHUNK = NC_PAD // 128  # 272

    def leaky_act(engine, out_ap, in_ap, bias=0.0, scale=1.0):
        """leaky_relu(scale*x + bias) on ScalarE (Lrelu) with DVE fallback."""
        if USE_LRELU:
            nc.scalar.activation(out_ap, in_ap, AF.Prelu, bias=bias, scale=scale,
                                 alpha=0.1)
        else:
            raise NotImplementedError

    def leaky_dve(pool, out_ap, psum_ap):
        """leaky from PSUM: only one PSUM read port, so use ScalarE Lrelu."""
        nc.scalar.activation(out_ap, psum_ap, AF.Prelu, alpha=0.1)

    with tile.TileContext(nc) as tc:
        with (
            tc.tile_pool(name="const", bufs=1) as cpool,
            tc.tile_pool(name="persist", bufs=1) as ppool,
            tc.tile_pool(name="psA", bufs=3, space="PSUM") as psA,
            tc.tile_pool(name="psB", bufs=3, space="PSUM") as psB,
            tc.tile_pool(name="psacc", bufs=2, space="PSUM") as psacc,
        ):
            # ---- load constants ----------------------------------------
            W = {}
            # [256,X] weights live as [128, 2, X]
            for name, cols, ddt in [
                ("w_caff", H, dt.bfloat16), ("w_csup", H, dt.bfloat16),
                ("w_paff", H, dt.bfloat16), ("w_aW1a", H, dt.bfloat16),
                ("w_rW1a", H, dt.bfloat16), ("w_W1b", 2 * H, dt.bfloat16),
                ("w_W1c", 2 * H, dt.bfloat16), ("w_aW2", 1, dt.bfloat16),
                ("w_rW2", H, dt.float32), ("w_oW1", H, dt.float32),
                ("w_oW2", 1, dt.float32),
            ]:
                t = cpool.tile([128, 2, cols], ddt, name=f"sb_{name}")
                nc.sync.dma_start(t[:], d_w[name].rearrange("(k p) n -> p k n", p=128))
                W[name] = t
            for name, cols, ddt in [
                ("b1_row", 2 * H, dt.bfloat16), ("b_aW2", 1, dt.bfloat16),
                ("b_csup_row", H, dt.bfloat16), ("rawb2_row", H, dt.float32),
                ("outb1_row", H, dt.float32), ("outb2_row", 1, dt.float32),
            ]:
                t = cpool.tile([1, cols], ddt, name=f"sb_{name}")
                nc.sync.dma_start(t[:], d_w[name][:])
                W[name] = t
            for name in ["b_caff", "b_paff"]:
                t = cpool.tile([128, 2, 1], dt.float32, name=f"sb_{name}")
                nc.sync.dma_start(t[:], d_w[name].rearrange("(k p) n -> p k n", p=128))
                W[name] = t
            ident = cpool.tile([128, 128], dt.float32, name="ident")
            nc.sync.dma_start(ident[:], d_w["id128"][:])
            ones_bf = cpool.tile([1, 128], dt.bfloat16, name="ones_bf")
            nc.vector.memset(ones_bf[:], 1.0)
            ones_f = cpool.tile([1, 128], dt.float32, name="ones_f")
            nc.vector.memset(ones_f[:], 1.0)
            # warm up the act table set (prelu+exp share exp_and_others) so
            # the implicit table-load doesn't ride a deeply-synced instruction
            warm = cpool.tile([1, 2], dt.float32, name="warm")
            nc.scalar.activation(warm[0:1, 0:1], ones_f[0:1, 0:1], AF.Prelu,
                                 alpha=0.1)
            nc.scalar.activation(warm[0:1, 1:2], ones_f[0:1, 0:1], AF.Exp)

            # ---- persistent tiles (embT allocated after prot phase) ----
            a_exp = ppool.tile([128, NCHUNK], dt.float32, name="a_exp")
            a_exp_bf = ppool.tile([128, NCHUNK], dt.bfloat16, name="a_exp_bf")
            alpha_cols = ppool.tile([128, NCHUNK], dt.float32, name="alpha_cols")
            poolT = [ppool.tile([128, GPC], dt.bfloat16, name=f"poolT{k}")
                     for k in range(2)]
            supeT = [ppool.tile([128, GPC], dt.bfloat16, name=f"supeT{k}")
                     for k in range(2)]
            wsumT = [ppool.tile([128, GPC], dt.float32, name=f"wsumT{k}")
                     for k in range(2)]
            g_gm = ppool.tile([128, NGROUP, 2 * H], dt.bfloat16, name="g_gm")
            asum = ppool.tile([128, NGROUP], dt.float32, name="asum")
            r_f = ppool.tile([128, NGROUP], dt.float32, name="r_f")
            rhi = ppool.tile([128, NGROUP], dt.bfloat16, name="rhi")
            rhi_f = ppool.tile([128, NGROUP], dt.float32, name="rhi_f")
            rlo = ppool.tile([128, NGROUP], dt.bfloat16, name="rlo")
            sg = ppool.tile([128, NGROUP], dt.float32, name="sg")
            sgT = ppool.tile([1, GPC], dt.float32, name="sgT")
            # ============================================================
            # Phase P: prot branch -> poolT (seg-max, GT layout, bf16)
            # ============================================================
            prpool = tc.alloc_tile_pool(name="protw", bufs=2)
            for sc in range(NSUPER):
                col0 = sc * P_SUPER
                pt0 = prpool.tile([128, P_SUPER], dt.bfloat16, name="pt0")
                pt1 = prpool.tile([128, P_SUPER], dt.bfloat16, name="pt1", tag="psA")
                nc.sync.dma_start(pt0[:], d_protT[0:128, col0:col0 + P_SUPER])
                nc.sync.dma_start(pt1[:], d_protT[128:256, col0:col0 + P_SUPER])
                e = [prpool.tile([128, P_SUPER], dt.bfloat16, name=f"pemb{m}")
                     for m in range(2)]
                for sub in range(P_SUPER // P_SUB):
                    s = sub * P_SUB
                    for m in range(2):
                        ps = psA.tile([128, P_SUB], dt.float32, name="ps_prot", tag="psA")
                        nc.tensor.matmul(ps[:], W["w_paff"][:, 0, m * 128:(m + 1) * 128],
                                         pt0[:, s:s + P_SUB], start=True, stop=False)
                        nc.tensor.matmul(ps[:], W["w_paff"][:, 1, m * 128:(m + 1) * 128],
                                         pt1[:, s:s + P_SUB], start=False, stop=True)
                        leaky_act(nc.scalar, e[m][:, s:s + P_SUB], ps[:],
                                  bias=W["b_paff"][:, m, :])
                # tournament within super-chunk: [128, 32, 112] -> [128, 32]
                NG = P_SUPER // PSLOT  # graphs per super-chunk (16)
                for m in range(2):
                    e3 = e[m].rearrange("p (g s) -> p g s", s=PSLOT)
                    t56 = prpool.tile([128, NG, 56], dt.bfloat16, name="t56")
                    nc.vector.tensor_tensor(t56[:], e3[:, :, 0:56], e3[:, :, 56:112],
                                            op=OP.max)
                    t28 = prpool.tile([128, NG, 28], dt.bfloat16, name="t28")
                    nc.vector.tensor_tensor(t28[:], t56[:, :, 0:28], t56[:, :, 28:56],
                                            op=OP.max)
                    t14 = prpool.tile([128, NG, 14], dt.bfloat16, name="t14")
                    nc.vector.tensor_tensor(t14[:], t28[:, :, 0:14], t28[:, :, 14:28],
                                            op=OP.max)
                    t7 = prpool.tile([128, NG, 7], dt.bfloat16, name="t7")
                    nc.vector.tensor_tensor(t7[:], t14[:, :, 0:7], t14[:, :, 7:14],
                                            op=OP.max)
                    nc.vector.tensor_reduce(poolT[m][:, sc * NG:(sc + 1) * NG], t7[:],
                                            axis=mybir.AxisListType.X, op=OP.max)
            prpool.release()
            lpool = tc.alloc_tile_pool(name="late", bufs=1)
            embT0 = lpool.tile([128, NC_PAD], dt.bfloat16, name="embT0")
            embT1 = lpool.tile([128, NC_PAD], dt.bfloat16, name="embT1")
            SCOLS = STAGE_BIGS * 512  # cols per DMA stage
            wpool = tc.alloc_tile_pool(name="workA", bufs=2)

            # ============================================================
            # Phase A: comp embeddings (c_aff) + sup_in -> supe
            # ============================================================
            for grp in range(NGROUP):
                psup = psacc.tile([128, H], dt.float32, name="psup", tag="psacc")
                for st in range(STAGES):
                    base = grp * GRP_SLOTS + st * SCOLS
                    ct0 = wpool.tile([128, SCOLS], dt.bfloat16, name="ct0")
                    ct1 = wpool.tile([128, SCOLS], dt.bfloat16, name="ct1")
                    nc.sync.dma_start(ct0[:], d_compT[0:128, base:base + SCOLS])
                    nc.sync.dma_start(ct1[:], d_compT[128:256, base:base + SCOLS])
                    gn = wpool.tile([128, SCOLS // 128, GRP_G], dt.bfloat16,
                                    name="gn")
                    nc.sync.dma_start(
                        gn[:],
                        d_Gn[base:base + SCOLS].rearrange("(s p) g -> p s g", p=128))

                    for bci in range(STAGE_BIGS):
                        o = bci * 512
                        c0 = base + o
                        # c_aff (TL) -> embT (persist)
                        for m in range(2):
                            pe = psA.tile([128, 512], dt.float32, name="pe_caff",
                                          tag="psA")
                            nc.tensor.matmul(pe[:],
                                             W["w_caff"][:, 0, m * 128:(m + 1) * 128],
                                             ct0[:, o:o + 512], start=True, stop=False)
                            nc.tensor.matmul(pe[:],
                                             W["w_caff"][:, 1, m * 128:(m + 1) * 128],
                                             ct1[:, o:o + 512], start=False, stop=True)
                            dst = embT0 if m == 0 else embT1
                            leaky_act(nc.scalar, dst[:, c0:c0 + 512], pe[:],
                                      bias=W["b_caff"][:, m, :])

                        for sub in range(4):
                            s = o + sub * 128
                            # sup_in (NM)
                            psin = psA.tile([128, H], dt.float32, name="psin",
                                            tag="psA")
                            nc.tensor.matmul(psin[:], ct0[:, s:s + 128],
                                             W["w_csup"][:, 0, :],
                                             start=True, stop=False)
                            nc.tensor.matmul(psin[:], ct1[:, s:s + 128],
                                             W["w_csup"][:, 1, :],
                                             start=False, stop=False)
                            nc.tensor.matmul(psin[:], ones_bf[:, 0:128],
                                             W["b_csup_row"][:],
                                             start=False, stop=True)
                            supin = wpool.tile([128, H], dt.bfloat16, name="supin")
                            leaky_dve(wpool, supin[:], psin[:])
                            # seg-sum: supe
                            nc.tensor.matmul(
                                psup[:], gn[:, bci * 4 + sub, :], supin[:],
                                start=(st == 0 and bci == 0 and sub == 0),
                                stop=(st == STAGES - 1 and
                                      bci == STAGE_BIGS - 1 and sub == 3))
                # group done: evacuate supe, transpose
                sup_gm = wpool.tile([128, H], dt.float32, name="sup_gm")
                nc.scalar.activation(sup_gm[:], psup[:], AF.Copy)
                for k in range(2):
                    ptr = psB.tile([128, 128], dt.float32, name="ptr_supe", tag="psB")
                    nc.tensor.transpose(ptr[:], sup_gm[:, k * 128:(k + 1) * 128],
                                        ident[:])
                    nc.vector.tensor_copy(
                        supeT[k][:, grp * 128:(grp + 1) * 128], ptr[:])

            # ---- g_both = supe@W1b + pool@W1c + b1 (GM, bf16) ----------
            for blk in range(NGROUP):
                pg = psA.tile([128, 2 * H], dt.float32, name="pg_gboth", tag="psA")
                for k in range(2):
                    nc.tensor.matmul(pg[:], supeT[k][:, blk * 128:(blk + 1) * 128],
                                     W["w_W1b"][:, k, :], start=(k == 0), stop=False)
                for k in range(2):
                    nc.tensor.matmul(pg[:], poolT[k][:, blk * 128:(blk + 1) * 128],
                                     W["w_W1c"][:, k, :], start=False, stop=False)
                nc.tensor.matmul(pg[:], ones_bf[:, 0:128], W["b1_row"][:],
                                 start=False, stop=True)
                nc.vector.tensor_copy(g_gm[:, blk, :], pg[:])

            wpool.release()
            wpool = tc.alloc_tile_pool(name="workB", bufs=2)
            # ============================================================
            # Phase B: h_alpha -> prealpha -> a_exp -> a_sum
            # ============================================================
            for grp in range(NGROUP):
                pasum = psacc.tile([128, 1], dt.float32, name="pasum", tag="psacc")
                for st in range(STAGES):
                    base = grp * GRP_SLOTS + st * SCOLS
                    bc0 = grp * BIG_PER_GROUP + st * STAGE_BIGS
                    ggs = wpool.tile([128, STAGE_BIGS, 512], dt.bfloat16, name="gg")
                    nc.sync.dma_start(
                        ggs[:], d_Gg[bc0:bc0 + STAGE_BIGS].rearrange("c p n -> p c n"))
                    gn = wpool.tile([128, SCOLS // 128, GRP_G], dt.bfloat16,
                                    name="gnB0")
                    nc.sync.dma_start(
                        gn[:],
                        d_Gn[base:base + SCOLS].rearrange("(s p) g -> p s g", p=128))

                    for bci in range(STAGE_BIGS):
                        o = bci * 512
                        c0 = base + o
                        # h_alpha (TL): emb @ aW1a + gather(g_alpha incl. bias)
                        haT = [wpool.tile([128, 512], dt.bfloat16, name=f"haT{m}")
                               for m in range(2)]
                        for m in range(2):
                            ph = psB.tile([128, 512], dt.float32, name="ph_hal",
                                          tag="psB")
                            nc.tensor.matmul(ph[:],
                                             W["w_aW1a"][:, 0, m * 128:(m + 1) * 128],
                                             embT0[:, c0:c0 + 512],
                                             start=True, stop=False)
                            nc.tensor.matmul(ph[:],
                                             W["w_aW1a"][:, 1, m * 128:(m + 1) * 128],
                                             embT1[:, c0:c0 + 512],
                                             start=False, stop=False)
                            nc.tensor.matmul(
                                ph[:], g_gm[:, grp, H + m * 128:H + (m + 1) * 128],
                                ggs[:, bci, :], start=False, stop=True)
                            leaky_dve(wpool, haT[m][:], ph[:])

                        for sub in range(4):
                            gc = (bc0 + bci) * 4 + sub
                            s = sub * 128
                            ppa = psA.tile([128, 1], dt.float32, name="ppa", tag="psA")
                            nc.tensor.matmul(ppa[:], haT[0][:, s:s + 128],
                                             W["w_aW2"][:, 0, :],
                                             start=True, stop=False)
                            nc.tensor.matmul(ppa[:], haT[1][:, s:s + 128],
                                             W["w_aW2"][:, 1, :],
                                             start=False, stop=False)
                            nc.tensor.matmul(ppa[:], ones_bf[:, 0:128],
                                             W["b_aW2"][:], start=False, stop=True)
                            nc.scalar.activation(a_exp[:, gc:gc + 1], ppa[:], AF.Exp)
                            nc.vector.tensor_copy(a_exp_bf[:, gc:gc + 1],
                                                  a_exp[:, gc:gc + 1])
                            nc.tensor.matmul(
                                pasum[:], gn[:, bci * 4 + sub, :],
                                a_exp_bf[:, gc:gc + 1],
                                start=(st == 0 and bci == 0 and sub == 0),
                                stop=(st == STAGES - 1 and
                                      bci == STAGE_BIGS - 1 and sub == 3))
                nc.vector.tensor_copy(asum[:, grp:grp + 1], pasum[:])

            # ---- per-graph mid layer: r = 1/(asum+eps), s_g = asum*r ---
            asum_eps = ppool.tile([128, NGROUP], dt.float32, name="asum_eps")
            nc.vector.tensor_scalar_add(asum_eps[:], asum[:], 1e-6)
            nc.vector.reciprocal(r_f[:], asum_eps[:])
            nc.vector.tensor_copy(rhi[:], r_f[:])
            nc.vector.tensor_copy(rhi_f[:], rhi[:])
            nc.vector.tensor_sub(rlo[:], r_f[:], rhi_f[:])
            nc.vector.tensor_mul(sg[:], asum[:], r_f[:])
            for k in range(NGROUP):
                p_sg = psB.tile([1, 128], dt.float32, name="p_sg", tag="psB")
                nc.tensor.transpose(p_sg[:], sg[:, k:k + 1], ident[:])
                nc.vector.tensor_copy(sgT[0:1, k * 128:(k + 1) * 128], p_sg[:])

            wpool.release()
            wpool = tc.alloc_tile_pool(name="workC", bufs=2)
            tpool = tc.alloc_tile_pool(name="tail", bufs=1)
            vecT = [tpool.tile([128, GPC], dt.float32, name=f"vecT{k}")
                    for k in range(2)]
            t1T = [tpool.tile([128, GPC], dt.float32, name=f"t1T{k}")
                   for k in range(2)]
            # ============================================================
            # Phase C: h_raw, alpha, z, wsum
            # ============================================================
            for grp in range(NGROUP):
                pw = psacc.tile([128, H], dt.float32, name="pw", tag="psacc")
                for st in range(STAGES):
                    base = grp * GRP_SLOTS + st * SCOLS
                    bc0 = grp * BIG_PER_GROUP + st * STAGE_BIGS
                    ggs = wpool.tile([128, STAGE_BIGS, 512], dt.bfloat16, name="ggB")
                    nc.sync.dma_start(
                        ggs[:], d_Gg[bc0:bc0 + STAGE_BIGS].rearrange("c p n -> p c n"))
                    gn = wpool.tile([128, SCOLS // 128, GRP_G], dt.bfloat16,
                                    name="gnB")
                    nc.sync.dma_start(
                        gn[:],
                        d_Gn[base:base + SCOLS].rearrange("(s p) g -> p s g", p=128))
                    for bsub in range(STAGE_BIGS * 4):
                        bci, sub = divmod(bsub, 4)
                        gc = (bc0 + bci) * 4 + sub
                        s = bci * 512 + sub * 128
                        phr = psB.tile([128, H + 1], dt.float32, name="phr", tag="psB")
                        nc.tensor.matmul(phr[:, 0:H], embT0[:, gc * 128:gc * 128 + 128],
                                         W["w_rW1a"][:, 0, :], start=True, stop=False)
                        nc.tensor.matmul(phr[:, 0:H], embT1[:, gc * 128:gc * 128 + 128],
                                         W["w_rW1a"][:, 1, :], start=False, stop=False)
                        nc.tensor.matmul(phr[:, 0:H], ggs[:, bci, sub * 128:sub * 128 + 128],
                                         g_gm[:, grp, 0:H], start=False, stop=True)
                        nc.tensor.matmul(phr[:, H:H + 1],
                                         ggs[:, bci, sub * 128:sub * 128 + 128],
                                         rhi[:, grp:grp + 1], start=True, stop=False)
                        nc.tensor.matmul(phr[:, H:H + 1],
                                         ggs[:, bci, sub * 128:sub * 128 + 128],
                                         rlo[:, grp:grp + 1], start=False, stop=True)
                        nc.vector.tensor_mul(alpha_cols[:, gc:gc + 1],
                                             a_exp[:, gc:gc + 1], phr[:, H:H + 1])
                        z = wpool.tile([128, H], dt.bfloat16, name="z")
                        if USE_LRELU:
                            nc.scalar.activation(z[:], phr[:, 0:H], AF.Prelu,
                                                 scale=alpha_cols[:, gc:gc + 1],
                                                 alpha=0.1)
                        else:
                            raise NotImplementedError
                        nc.tensor.matmul(
                            pw[:], gn[:, bci * 4 + sub, :], z[:],
                            start=(st == 0 and bsub == 0),
                            stop=(st == STAGES - 1 and bsub == STAGE_BIGS * 4 - 1))
                wsum_gm = wpool.tile([128, H], dt.float32, name="wsum_gm")
                nc.scalar.activation(wsum_gm[:], pw[:], AF.Copy)
                for k in range(2):
                    ptr = psB.tile([128, 128], dt.float32, name="ptr_wsum", tag="psB")
                    nc.tensor.transpose(ptr[:], wsum_gm[:, k * 128:(k + 1) * 128],
                                        ident[:])
                    nc.vector.tensor_copy(
                        wsumT[k][:, grp * 128:(grp + 1) * 128], ptr[:])

            # ============================================================
            # Tail: vector & affinity (f32)
            # ============================================================
            for blk in range(NGROUP):
                pv = psA.tile([128, H], dt.float32, name="pv", tag="psA")
                nc.tensor.matmul(pv[:], wsumT[0][:, blk * 128:(blk + 1) * 128],
                                 W["w_rW2"][:, 0, :], start=True, stop=False)
                nc.tensor.matmul(pv[:], wsumT[1][:, blk * 128:(blk + 1) * 128],
                                 W["w_rW2"][:, 1, :], start=False, stop=False)
                nc.tensor.matmul(pv[:], sgT[0:1, blk * 128:(blk + 1) * 128],
                                 W["rawb2_row"][:], start=False, stop=True)
                vec_gm = wpool.tile([128, H], dt.float32, name="vec_gm")
                nc.scalar.activation(vec_gm[:], pv[:], AF.Copy)
                nc.sync.dma_start(d_vec[blk * 128:(blk + 1) * 128, :], vec_gm[:])
                for k in range(2):
                    ptr = psB.tile([128, 128], dt.float32, name="ptr_vec", tag="psB")
                    nc.tensor.transpose(ptr[:], vec_gm[:, k * 128:(k + 1) * 128],
                                        ident[:])
                    nc.vector.tensor_copy(vecT[k][:, blk * 128:(blk + 1) * 128], ptr[:])
            for blk in range(NGROUP):
                pt1 = psA.tile([128, H], dt.float32, name="pt1", tag="psA")
                nc.tensor.matmul(pt1[:], vecT[0][:, blk * 128:(blk + 1) * 128],
                                 W["w_oW1"][:, 0, :], start=True, stop=False)
                nc.tensor.matmul(pt1[:], vecT[1][:, blk * 128:(blk + 1) * 128],
                                 W["w_oW1"][:, 1, :], start=False, stop=False)
                nc.tensor.matmul(pt1[:], ones_f[:, 0:128], W["outb1_row"][:],
                                 start=False, stop=True)
                t1_gm = wpool.tile([128, H], dt.float32, name="t1_gm")
                leaky_dve(wpool, t1_gm[:], pt1[:])
                for k in range(2):
                    ptr = psB.tile([128, 128], dt.float32, name="ptr_t1", tag="psB")
                    nc.tensor.transpose(ptr[:], t1_gm[:, k * 128:(k + 1) * 128],
                                        ident[:])
                    nc.vector.tensor_copy(t1T[k][:, blk * 128:(blk + 1) * 128], ptr[:])
            aff_sb = ppool.tile([128, NGROUP], dt.float32, name="aff_sb")
            for blk in range(NGROUP):
                pa = psB.tile([128, 1], dt.float32, name="pa", tag="psB")
                nc.tensor.matmul(pa[:], t1T[0][:, blk * 128:(blk + 1) * 128],
                                 W["w_oW2"][:, 0, :], start=True, stop=False)
                nc.tensor.matmul(pa[:], t1T[1][:, blk * 128:(blk + 1) * 128],
                                 W["w_oW2"][:, 1, :], start=False, stop=False)
                nc.tensor.matmul(pa[:], ones_f[:, 0:128], W["outb2_row"][:],
                                 start=False, stop=True)
                nc.scalar.activation(aff_sb[:, blk:blk + 1], pa[:], AF.Copy)

            nc.sync.dma_start(d_alpha[:], alpha_cols[:])
            nc.sync.dma_start(d_aff.rearrange("(b p) one -> p (b one)", p=128),
                              aff_sb[:])
            tpool.release()
            wpool.release()
            lpool.release()

    if not nc.is_finalized():
        nc.finalize()
    return nc


_PROGRAM_CACHE = {}
TRACE = False
LAST_RESULTS = None
LAST_EXEC_WALL = None


def kernel(comp_feature, prot_feature, batch_comp, batch_prot,
           c_aff_W, c_aff_b, c_sup_W, c_sup_b, p_aff_W, p_aff_b,
           raw_W1, raw_b1, raw_W2, raw_b2,
           alpha_W1, alpha_b1, alpha_W2, alpha_b2,
           out_W1, out_b1, out_W2, out_b2):
    from concourse.bass_utils import run_bass_kernel_spmd

    inp = dict(c_aff_W=c_aff_W, c_aff_b=c_aff_b, c_sup_W=c_sup_W, c_sup_b=c_sup_b,
               p_aff_W=p_aff_W, p_aff_b=p_aff_b, raw_W1=raw_W1, raw_b1=raw_b1,
               raw_W2=raw_W2, raw_b2=raw_b2, alpha_W1=alpha_W1, alpha_b1=alpha_b1,
               alpha_W2=alpha_W2, alpha_b2=alpha_b2, out_W1=out_W1, out_b1=out_b1,
               out_W2=out_W2, out_b2=out_b2)
    inp = {k: np.asarray(v) for k, v in inp.items()}
    comp_feature = np.asarray(comp_feature, np.float32)
    prot_feature = np.asarray(prot_feature, np.float32)
    batch_comp = np.asarray(batch_comp, np.int64)
    batch_prot = np.asarray(batch_prot, np.int64)

    # dummy prot feature vector: p_aff_W^T v + b = -1e4 -> emb = -1000
    v_dummy = np.linalg.solve(np.asarray(inp["p_aff_W"], np.float64).T,
                              (-1e4 - np.asarray(inp["p_aff_b"], np.float64)))
    chk = np.asarray(inp["p_aff_W"], np.float32).T.astype(np.float32) @ \
        v_dummy.astype(BF16).astype(np.float32) + np.asarray(inp["p_aff_b"], np.float32)
    assert chk.max() < -5e3, f"v_dummy check failed: {chk.max()}"

    wmaps = _weights_maps(inp)

    in_maps = []
    slot_maps = []
    for core in range(NCORES):
        part, slot2node = _prep_core(core, comp_feature, prot_feature,
                                     batch_comp, batch_prot, v_dummy)
        part.update(wmaps)
        in_maps.append(part)
        slot_maps.append(slot2node)

    if "nc" not in _PROGRAM_CACHE:
        _PROGRAM_CACHE["nc"] = _build_program()
    nc = _PROGRAM_CACHE["nc"]

    import time as _time
    global LAST_RESULTS, LAST_EXEC_WALL
    t0 = _time.time()
    res = run_bass_kernel_spmd(nc, in_maps, core_ids=list(range(NCORES)),
                               trace=TRACE)
    LAST_EXEC_WALL = _time.time() - t0
    LAST_RESULTS = res
    results = res.results

    vector = np.zeros((B, H), np.float32)
    alpha = np.zeros((N, 1), np.float32)
    affinity = np.zeros((B, 1), np.float32)
    for core in range(NCORES):
        r = results[core]
        vector[core * GPC:(core + 1) * GPC] = r["out_vector"]
        affinity[core * GPC:(core + 1) * GPC] = r["out_aff"]
        a_cols = np.asarray(r["out_alpha"], np.float32)     # [128, 272]
        a_flat = a_cols.T.reshape(-1)                       # slot-ordered
        s2n = slot_maps[core]
        real = s2n >= 0
        alpha[s2n[real], 0] = a_flat[real]
    return vector, alpha, affinity


# revision 36
# speedup vs baseline: 15601.9586x; 15601.9586x over previous
"""Trainium2 Bass kernel for AffinityNeuralNetworkCliffNet (gnn_message_passing).

Strategy: data-parallel over graphs (512 graphs/core on 8 cores). Host does
layout prep only (shard / pad / transpose / one-hot segment matrices / bf16
cast); all FLOPs over node data run on-device.

Per-core layout (identical program on all cores, data differs):
  - comp nodes grouped into 4 groups of 128 graphs, each group padded to 9216
    node slots (72 chunks of 128) -> NC_PAD = 36864.
  - prot nodes slotted: 112 slots per graph (dummy slots get a feature vector v
    with p_aff_W^T v + p_aff_b = -1e4, so emb ~ -1000 = -inf for seg-max).
  - Segment sums / gathers are one-hot matmuls (Gn: [node,128-graph-group],
    Gg: [128-graph-group, node]); seg-max is a contiguous-slot max tournament.
"""

import sys
import numpy as np

sys.path.insert(0, "/opt/trn_rl_repo")

import ml_dtypes  # noqa: E402

BF16 = ml_dtypes.bfloat16

B = 4096
H = 256
N = 262144
NCORES = 8
GPC = 512              # graphs per core
NGROUP = 4             # graph groups per core (128 graphs each)
GRP_G = 128            # graphs per group
GRP_SLOTS = 9216       # comp node slots per group (72 chunks of 128)
NC_PAD = NGROUP * GRP_SLOTS          # 36864 comp slots/core
NBIG = NC_PAD // 512                 # 72 big chunks of 512
BIG_PER_GROUP = GRP_SLOTS // 512     # 18
STAGE_BIGS = 3         # big-chunks per DMA stage
STAGES = BIG_PER_GROUP // STAGE_BIGS   # 6
PSLOT = 112            # prot slots per graph
NP_PAD = GPC * PSLOT   # 57344 prot slots/core
P_SUPER = 7168         # prot super-chunk cols (64 graphs)
NSUPER = NP_PAD // P_SUPER           # 8
P_SUB = 448            # prot matmul sub-chunk (4 graphs)

USE_LRELU = True       # ScalarE Lrelu(alpha=0.1); fallback: DVE 2-op leaky


# ----------------------------------------------------------------------------
# Host-side prep
# ----------------------------------------------------------------------------

def _prep_core(core, comp_feature, prot_feature, batch_comp, batch_prot, v_dummy):
    """Build one core's input arrays. Returns (in_map_part, meta)."""
    g0 = core * GPC

    # ---- comp side: group-padded layout --------------------------------
    compT = np.zeros((H, NC_PAD), dtype=BF16)
    Gn = np.zeros((NC_PAD, GRP_G), dtype=BF16)
    Gg = np.zeros((NBIG, GRP_G, 512), dtype=BF16)
    # map: padded slot -> original node index (or -1)
    slot2node = np.full(NC_PAD, -1, dtype=np.int64)

    for grp in range(NGROUP):
        glo = g0 + grp * GRP_G
        ghi = glo + GRP_G
        lo = np.searchsorted(batch_comp, glo, side="left")
        hi = np.searchsorted(batch_comp, ghi, side="left")
        n = hi - lo
        assert n <= GRP_SLOTS, f"comp group overflow: {n} > {GRP_SLOTS}"
        base = grp * GRP_SLOTS
        compT[:, base:base + n] = comp_feature[lo:hi].T.astype(BF16)
        slot2node[base:base + n] = np.arange(lo, hi)
        lids = batch_comp[lo:hi] - glo          # 0..127 within group
        Gn[base + np.arange(n), lids] = 1.0
        # Gg planes for this group's big chunks
        for bc in range(grp * BIG_PER_GROUP, (grp + 1) * BIG_PER_GROUP):
            s = bc * 512
            js = np.arange(512)
            sl = slot2node[s:s + 512]
            real = sl >= 0
            if real.any():
                lid = batch_comp[sl[real]] - glo
                Gg[bc, lid, js[real]] = 1.0

    # ---- prot side: 112-slot layout ------------------------------------
    protT = np.empty((H, NP_PAD), dtype=BF16)
    protT[:] = v_dummy[:, None].astype(BF16)
    for g in range(GPC):
        lo = np.searchsorted(batch_prot, g0 + g, side="left")
        hi = np.searchsorted(batch_prot, g0 + g + 1, side="left")
        n = hi - lo
        assert n <= PSLOT, f"prot graph overflow: {n} > {PSLOT}"
        protT[:, g * PSLOT:g * PSLOT + n] = prot_feature[lo:hi].T.astype(BF16)

    part = {
        "compT": compT,
        "protT": protT,
        "Gn": Gn,
        "Gg": Gg,
    }
    return part, slot2node


def _weights_maps(inp):
    """Shared (replicated) weight tensors."""
    f32 = np.float32
    def bf(x):
        return np.ascontiguousarray(x, dtype=np.float32).astype(BF16)

    w = {}
    w["w_caff"] = bf(inp["c_aff_W"])                      # [256,256]
    w["w_csup"] = bf(inp["c_sup_W"])
    w["w_paff"] = bf(inp["p_aff_W"])
    w["w_aW1a"] = bf(inp["alpha_W1"][:H])                 # lhsT for TL h_alpha
    w["w_rW1a"] = bf(inp["raw_W1"][:H])                   # rhs for NM h_raw
    w["w_W1b"] = bf(np.concatenate([inp["raw_W1"][H:2 * H],
                                    inp["alpha_W1"][H:2 * H]], axis=1))  # [256,512]
    w["w_W1c"] = bf(np.concatenate([inp["raw_W1"][2 * H:],
                                    inp["alpha_W1"][2 * H:]], axis=1))   # [256,512]
    w["b1_row"] = bf(np.concatenate([inp["raw_b1"], inp["alpha_b1"]])[None, :])  # [1,512]
    w["w_aW2"] = bf(inp["alpha_W2"])                      # [256,1]
    w["b_aW2"] = bf(np.asarray(inp["alpha_b2"]).reshape(1, 1))
    w["b_csup_row"] = bf(np.asarray(inp["c_sup_b"])[None, :])   # [1,256]
    w["b_caff"] = np.ascontiguousarray(np.asarray(inp["c_aff_b"], f32).reshape(H, 1))
    w["b_paff"] = np.ascontiguousarray(np.asarray(inp["p_aff_b"], f32).reshape(H, 1))
    w["w_rW2"] = np.ascontiguousarray(inp["raw_W2"], dtype=f32)       # [256,256]
    w["rawb2_row"] = np.ascontiguousarray(np.asarray(inp["raw_b2"], f32)[None, :])
    w["w_oW1"] = np.ascontiguousarray(inp["out_W1"], dtype=f32)
    w["outb1_row"] = np.ascontiguousarray(np.asarray(inp["out_b1"], f32)[None, :])
    w["w_oW2"] = np.ascontiguousarray(inp["out_W2"], dtype=f32)       # [256,1]
    w["outb2_row"] = np.ascontiguousarray(np.asarray(inp["out_b2"], f32).reshape(1, 1))
    w["id128"] = np.eye(128, dtype=f32)
    return w


# ----------------------------------------------------------------------------
# Device program (identical for all cores)
# ----------------------------------------------------------------------------

def _build_program():
    import concourse.bass as bass  # noqa: F401
    import concourse.bacc as bacc
    import concourse.mybir as mybir
    import concourse.tile as tile

    dt = mybir.dt
    AF = mybir.ActivationFunctionType
    OP = mybir.AluOpType

    nc = bacc.Bacc("TRN2", target_bir_lowering=False)

    # ---- dram tensors ---------------------------------------------------
    d_compT = nc.dram_tensor("compT", [H, NC_PAD], dt.bfloat16, kind="ExternalInput")
    d_protT = nc.dram_tensor("protT", [H, NP_PAD], dt.bfloat16, kind="ExternalInput")
    d_Gn = nc.dram_tensor("Gn", [NC_PAD, GRP_G], dt.bfloat16, kind="ExternalInput")
    d_Gg = nc.dram_tensor("Gg", [NBIG, GRP_G, 512], dt.bfloat16, kind="ExternalInput")

    d_w = {}
    for name, shape, ddt in [
        ("w_caff", [H, H], dt.bfloat16), ("w_csup", [H, H], dt.bfloat16),
        ("w_paff", [H, H], dt.bfloat16), ("w_aW1a", [H, H], dt.bfloat16),
        ("w_rW1a", [H, H], dt.bfloat16), ("w_W1b", [H, 2 * H], dt.bfloat16),
        ("w_W1c", [H, 2 * H], dt.bfloat16), ("b1_row", [1, 2 * H], dt.bfloat16),
        ("w_aW2", [H, 1], dt.bfloat16), ("b_aW2", [1, 1], dt.bfloat16),
        ("b_csup_row", [1, H], dt.bfloat16),
        ("b_caff", [H, 1], dt.float32), ("b_paff", [H, 1], dt.float32),
        ("w_rW2", [H, H], dt.float32), ("rawb2_row", [1, H], dt.float32),
        ("w_oW1", [H, H], dt.float32), ("outb1_row", [1, H], dt.float32),
        ("w_oW2", [H, 1], dt.float32), ("outb2_row", [1, 1], dt.float32),
        ("id128", [128, 128], dt.float32),
    ]:
        d_w[name] = nc.dram_tensor(name, shape, ddt, kind="ExternalInput")

    d_alpha = nc.dram_tensor("out_alpha", [128, NC_PAD // 128], dt.float32,
                             kind="ExternalOutput")
    d_vec = nc.dram_tensor("out_vector", [GPC, H], dt.float32, kind="ExternalOutput")
    d_aff = nc.dram_tensor("out_aff", [GPC, 1], dt.float32, kind="ExternalOutput")

    NCHUNK = NC_PAD // 128  # 272

    def leaky_act(engine, out_ap, in_ap, bias=0.0, scale=1.0):
        """leaky_relu(scale*x + bias) on ScalarE (Lrelu) with DVE fallback."""
        if USE_LRELU:
            nc.scalar.activation(out_ap, in_ap, AF.Prelu, bias=bias, scale=scale,
                                 alpha=0.1)
        else:
            raise NotImplementedError

    def leaky_dve(pool, out_ap, psum_ap):
        """leaky from PSUM: only one PSUM read port, so use ScalarE Lrelu."""
        nc.scalar.activation(out_ap, psum_ap, AF.Prelu, alpha=0.1)

    with tile.TileContext(nc) as tc:
        with (
            tc.tile_pool(name="const", bufs=1) as cpool,
            tc.tile_pool(name="persist", bufs=1) as ppool,
            tc.tile_pool(name="psA", bufs=3, space="PSUM") as psA,
            tc.tile_pool(name="psB", bufs=3, space="PSUM") as psB,
            tc.tile_pool(name="psacc", bufs=2, space="PSUM") as psacc,
        ):
            # ---- load constants ----------------------------------------
            W = {}
            # [256,X] weights live as [128, 2, X]
            for name, cols, ddt in [
                ("w_caff", H, dt.bfloat16), ("w_csup", H, dt.bfloat16),
                ("w_paff", H, dt.bfloat16), ("w_aW1a", H, dt.bfloat16),
                ("w_rW1a", H, dt.bfloat16), ("w_W1b", 2 * H, dt.bfloat16),
                ("w_W1c", 2 * H, dt.bfloat16), ("w_aW2", 1, dt.bfloat16),
                ("w_rW2", H, dt.float32), ("w_oW1", H, dt.float32),
                ("w_oW2", 1, dt.float32),
            ]:
                t = cpool.tile([128, 2, cols], ddt, name=f"sb_{name}")
                nc.sync.dma_start(t[:], d_w[name].rearrange("(k p) n -> p k n", p=128))
                W[name] = t
            for name, cols, ddt in [
                ("b1_row", 2 * H, dt.bfloat16), ("b_aW2", 1, dt.bfloat16),
                ("b_csup_row", H, dt.bfloat16), ("rawb2_row", H, dt.float32),
                ("outb1_row", H, dt.float32), ("outb2_row", 1, dt.float32),
            ]:
                t = cpool.tile([1, cols], ddt, name=f"sb_{name}")
                nc.sync.dma_start(t[:], d_w[name][:])
                W[name] = t
            for name in ["b_caff", "b_paff"]:
                t = cpool.tile([128, 2, 1], dt.float32, name=f"sb_{name}")
                nc.sync.dma_start(t[:], d_w[name].rearrange("(k p) n -> p k n", p=128))
                W[name] = t
            ident = cpool.tile([128, 128], dt.float32, name="ident")
            nc.sync.dma_start(ident[:], d_w["id128"][:])
            ones_bf = cpool.tile([1, 128], dt.bfloat16, name="ones_bf")
            nc.vector.memset(ones_bf[:], 1.0)
            ones_f = cpool.tile([1, 128], dt.float32, name="ones_f")
            nc.vector.memset(ones_f[:], 1.0)
            # warm up the act table set (prelu+exp share exp_and_others) so
            # the implicit table-load doesn't ride a deeply-synced instruction
            warm = cpool.tile([1, 2], dt.float32, name="warm")
            nc.scalar.activation(warm[0:1, 0:1], ones_f[0:1, 0:1], AF.Prelu,
                                 alpha=0.1)
            nc.scalar.activation(warm[0:1, 1:2], ones_f[0:1, 0:1], AF.Exp)

            # ---- persistent tiles (embT allocated after prot phase) ----
            a_exp = ppool.tile([128, NCHUNK], dt.float32, name="a_exp")
            a_exp_bf = ppool.tile([128, NCHUNK], dt.bfloat16, name="a_exp_bf")
            alpha_cols = ppool.tile([128, NCHUNK], dt.float32, name="alpha_cols")
            poolT = [ppool.tile([128, GPC], dt.bfloat16, name=f"poolT{k}")
                     for k in range(2)]
            supeT = [ppool.tile([128, GPC], dt.bfloat16, name=f"supeT{k}")
                     for k in range(2)]
            wsumT = [ppool.tile([128, GPC], dt.float32, name=f"wsumT{k}")
                     for k in range(2)]
            g_gm = ppool.tile([128, NGROUP, 2 * H], dt.bfloat16, name="g_gm")
            asum = ppool.tile([128, NGROUP], dt.float32, name="asum")
            r_f = ppool.tile([128, NGROUP], dt.float32, name="r_f")
            rhi = ppool.tile([128, NGROUP], dt.bfloat16, name="rhi")
            rhi_f = ppool.tile([128, NGROUP], dt.float32, name="rhi_f")
            rlo = ppool.tile([128, NGROUP], dt.bfloat16, name="rlo")
            sg = ppool.tile([128, NGROUP], dt.float32, name="sg")
            sgT = ppool.tile([1, GPC], dt.float32, name="sgT")
            # ============================================================
            # Phase P: prot branch -> poolT (seg-max, GT layout, bf16)
            # ============================================================
            prpool = tc.alloc_tile_pool(name="protw", bufs=2)
            for sc in range(NSUPER):
                col0 = sc * P_SUPER
                pt0 = prpool.tile([128, P_SUPER], dt.bfloat16, name="pt0")
                pt1 = prpool.tile([128, P_SUPER], dt.bfloat16, name="pt1", tag="psA")
                nc.sync.dma_start(pt0[:], d_protT[0:128, col0:col0 + P_SUPER])
                nc.sync.dma_start(pt1[:], d_protT[128:256, col0:col0 + P_SUPER])
                e = [prpool.tile([128, P_SUPER], dt.bfloat16, name=f"pemb{m}")
                     for m in range(2)]
                for sub in range(P_SUPER // P_SUB):
                    s = sub * P_SUB
                    for m in range(2):
                        ps = psA.tile([128, P_SUB], dt.float32, name="ps_prot", tag="psA")
                        nc.tensor.matmul(ps[:], W["w_paff"][:, 0, m * 128:(m + 1) * 128],
                                         pt0[:, s:s + P_SUB], start=True, stop=False)
                        nc.tensor.matmul(ps[:], W["w_paff"][:, 1, m * 128:(m + 1) * 128],
                                         pt1[:, s:s + P_SUB], start=False, stop=True)
                        leaky_act(nc.scalar, e[m][:, s:s + P_SUB], ps[:],
                                  bias=W["b_paff"][:, m, :])
                # tournament within super-chunk: [128, 32, 112] -> [128, 32]
                NG = P_SUPER // PSLOT  # graphs per super-chunk (16)
                for m in range(2):
                    e3 = e[m].rearrange("p (g s) -> p g s", s=PSLOT)
                    t56 = prpool.tile([128, NG, 56], dt.bfloat16, name="t56")
                    nc.vector.tensor_tensor(t56[:], e3[:, :, 0:56], e3[:, :, 56:112],
                                            op=OP.max)
                    t28 = prpool.tile([128, NG, 28], dt.bfloat16, name="t28")
                    nc.vector.tensor_tensor(t28[:], t56[:, :, 0:28], t56[:, :, 28:56],
                                            op=OP.max)
                    t14 = prpool.tile([128, NG, 14], dt.bfloat16, name="t14")
                    nc.vector.tensor_tensor(t14[:], t28[:, :, 0:14], t28[:, :, 14:28],
                                            op=OP.max)
                    t7 = prpool.tile([128, NG, 7], dt.bfloat16, name="t7")
                    nc.vector.tensor_tensor(t7[:], t14[:, :, 0:7], t14[:, :, 7:14],
                                            op=OP.max)
                    nc.vector.tensor_reduce(poolT[m][:, sc * NG:(sc + 1) * NG], t7[:],
                                            axis=mybir.AxisListType.X, op=OP.max)
            prpool.release()
            lpool = tc.alloc_tile_pool(name="late", bufs=1)
            embT0 = lpool.tile([128, NC_PAD], dt.bfloat16, name="embT0")
            embT1 = lpool.tile([128, NC_PAD], dt.bfloat16, name="embT1")
            SCOLS = STAGE_BIGS * 512  # cols per DMA stage
            wpool = tc.alloc_tile_pool(name="workA", bufs=2)

            # ============================================================
            # Phase A: comp embeddings (c_aff) + sup_in -> supe
            # ============================================================
            for grp in range(NGROUP):
                psup = psacc.tile([128, H], dt.float32, name="psup", tag="psacc")
                for st in range(STAGES):
                    base = grp * GRP_SLOTS + st * SCOLS
                    ct0 = wpool.tile([128, SCOLS], dt.bfloat16, name="ct0")
                    ct1 = wpool.tile([128, SCOLS], dt.bfloat16, name="ct1")
                    nc.sync.dma_start(ct0[:], d_compT[0:128, base:base + SCOLS])
                    nc.sync.dma_start(ct1[:], d_compT[128:256, base:base + SCOLS])
                    gn = wpool.tile([128, SCOLS // 128, GRP_G], dt.bfloat16,
                                    name="gn")
                    nc.sync.dma_start(
                        gn[:],
                        d_Gn[base:base + SCOLS].rearrange("(s p) g -> p s g", p=128))

                    for bci in range(STAGE_BIGS):
                        o = bci * 512
                        c0 = base + o
                        # c_aff (TL) -> embT (persist)
                        for m in range(2):
                            pe = psA.tile([128, 512], dt.float32, name="pe_caff",
                                          tag="psA")
                            nc.tensor.matmul(pe[:],
                                             W["w_caff"][:, 0, m * 128:(m + 1) * 128],
                                             ct0[:, o:o + 512], start=True, stop=False)
                            nc.tensor.matmul(pe[:],
                                             W["w_caff"][:, 1, m * 128:(m + 1) * 128],
                                             ct1[:, o:o + 512], start=False, stop=True)
                            dst = embT0 if m == 0 else embT1
                            leaky_act(nc.scalar, dst[:, c0:c0 + 512], pe[:],
                                      bias=W["b_caff"][:, m, :])

                        for sub in range(4):
                            s = o + sub * 128
                            # sup_in (NM)
                            psin = psA.tile([128, H], dt.float32, name="psin",
                                            tag="psA")
                            nc.tensor.matmul(psin[:], ct0[:, s:s + 128],
                                             W["w_csup"][:, 0, :],
                                             start=True, stop=False)
                            nc.tensor.matmul(psin[:], ct1[:, s:s + 128],
                                             W["w_csup"][:, 1, :],
                                             start=False, stop=False)
                            nc.tensor.matmul(psin[:], ones_bf[:, 0:128],
                                             W["b_csup_row"][:],
                                             start=False, stop=True)
                            supin = wpool.tile([128, H], dt.bfloat16, name="supin")
                            leaky_dve(wpool, supin[:], psin[:])
                            # seg-sum: supe
                            nc.tensor.matmul(
                                psup[:], gn[:, bci * 4 + sub, :], supin[:],
                                start=(st == 0 and bci == 0 and sub == 0),
                                stop=(st == STAGES - 1 and
                                      bci == STAGE_BIGS - 1 and sub == 3))
                # group done: evacuate supe, transpose
                sup_gm = wpool.tile([128, H], dt.float32, name="sup_gm")
                nc.scalar.activation(sup_gm[:], psup[:], AF.Copy)
                for k in range(2):
                    ptr = psB.tile([128, 128], dt.float32, name="ptr_supe", tag="psB")
                    nc.tensor.transpose(ptr[:], sup_gm[:, k * 128:(k + 1) * 128],
                                        ident[:])
                    nc.vector.tensor_copy(
                        supeT[k][:, grp * 128:(grp + 1) * 128], ptr[:])

            # ---- g_both = supe@W1b + pool@W1c + b1 (GM, bf16) ----------
            for blk in range(NGROUP):
                pg = psA.tile([128, 2 * H], dt.float32, name="pg_gboth", tag="psA")
                for k in range(2):
                    nc.tensor.matmul(pg[:], supeT[k][:, blk * 128:(blk + 1) * 128],
                                     W["w_W1b"][:, k, :], start=(k == 0), stop=False)
                for k in range(2):
                    nc.tensor.matmul(pg[:], poolT[k][:, blk * 128:(blk + 1) * 128],
                                     W["w_W1c"][:, k, :], start=False, stop=False)
                nc.tensor.matmul(pg[:], ones_bf[:, 0:128], W["b1_row"][:],
                                 start=False, stop=True)
                nc.vector.tensor_copy(g_gm[:, blk, :], pg[:])

            wpool.release()
            wpool = tc.alloc_tile_pool(name="workB", bufs=2)
            # ============================================================
            # Phase B: h_alpha -> prealpha -> a_exp -> a_sum
            # ============================================================
            for grp in range(NGROUP):
                pasum = psacc.tile([128, 1], dt.float32, name="pasum", tag="psacc")
                for st in range(STAGES):
                    base = grp * GRP_SLOTS + st * SCOLS
                    bc0 = grp * BIG_PER_GROUP + st * STAGE_BIGS
                    ggs = wpool.tile([128, STAGE_BIGS, 512], dt.bfloat16, name="gg")
                    nc.sync.dma_start(
                        ggs[:], d_Gg[bc0:bc0 + STAGE_BIGS].rearrange("c p n -> p c n"))
                    gn = wpool.tile([128, SCOLS // 128, GRP_G], dt.bfloat16,
                                    name="gnB0")
                    nc.sync.dma_start(
                        gn[:],
                        d_Gn[base:base + SCOLS].rearrange("(s p) g -> p s g", p=128))

                    for bci in range(STAGE_BIGS):
                        o = bci * 512
                        c0 = base + o
                        # h_alpha (TL): emb @ aW1a + gather(g_alpha incl. bias)
                        haT = [wpool.tile([128, 512], dt.bfloat16, name=f"haT{m}")
                               for m in range(2)]
                        for m in range(2):
                            ph = psB.tile([128, 512], dt.float32, name="ph_hal",
                                          tag="psB")
                            nc.tensor.matmul(ph[:],
                                             W["w_aW1a"][:, 0, m * 128:(m + 1) * 128],
                                             embT0[:, c0:c0 + 512],
                                             start=True, stop=False)
                            nc.tensor.matmul(ph[:],
                                             W["w_aW1a"][:, 1, m * 128:(m + 1) * 128],
                                             embT1[:, c0:c0 + 512],
                                             start=False, stop=False)
                            nc.tensor.matmul(
                                ph[:], g_gm[:, grp, H + m * 128:H + (m + 1) * 128],
                                ggs[:, bci, :], start=False, stop=True)
                            leaky_dve(wpool, haT[m][:], ph[:])

                        for sub in range(4):
                            gc = (bc0 + bci) * 4 + sub
                            s = sub * 128
                            ppa = psA.tile([128, 1], dt.float32, name="ppa", tag="psA")
                            nc.tensor.matmul(ppa[:], haT[0][:, s:s + 128],
                                             W["w_aW2"][:, 0, :],
                                             start=True, stop=False)
                            nc.tensor.matmul(ppa[:], haT[1][:, s:s + 128],
                                             W["w_aW2"][:, 1, :],
                                             start=False, stop=False)
                            nc.tensor.matmul(ppa[:], ones_bf[:, 0:128],
                                             W["b_aW2"][:], start=False, stop=True)
                            nc.scalar.activation(a_exp[:, gc:gc + 1], ppa[:], AF.Exp)
                            nc.vector.tensor_copy(a_exp_bf[:, gc:gc + 1],
                                                  a_exp[:, gc:gc + 1])
                            nc.tensor.matmul(
                                pasum[:], gn[:, bci * 4 + sub, :],
                                a_exp_bf[:, gc:gc + 1],
                                start=(st == 0 and bci == 0 and sub == 0),
                                stop=(st == STAGES - 1 and
                                      bci == STAGE_BIGS - 1 and sub == 3))
                nc.vector.tensor_copy(asum[:, grp:grp + 1], pasum[:])

            # ---- per-graph mid layer: r = 1/(asum+eps), s_g = asum*r ---
            asum_eps = ppool.tile([128, NGROUP], dt.float32, name="asum_eps")
            nc.vector.tensor_scalar_add(asum_eps[:], asum[:], 1e-6)
            nc.vector.reciprocal(r_f[:], asum_eps[:])
            nc.vector.tensor_copy(rhi[:], r_f[:])
            nc.vector.tensor_copy(rhi_f[:], rhi[:])
            nc.vector.tensor_sub(rlo[:], r_f[:], rhi_f[:])
            nc.vector.tensor_mul(sg[:], asum[:], r_f[:])
            for k in range(NGROUP):
                p_sg = psB.tile([1, 128], dt.float32, name="p_sg", tag="psB")
                nc.tensor.transpose(p_sg[:], sg[:, k:k + 1], ident[:])
                nc.vector.tensor_copy(sgT[0:1, k * 128:(k + 1) * 128], p_sg[:])

            wpool.release()
            wpool = tc.alloc_tile_pool(name="workC", bufs=2)
            tpool = tc.alloc_tile_pool(name="tail", bufs=1)
            vecT = [tpool.tile([128, GPC], dt.float32, name=f"vecT{k}")
                    for k in range(2)]
            t1T = [tpool.tile([128, GPC], dt.float32, name=f"t1T{k}")
                   for k in range(2)]
            # ============================================================
            # Phase C: h_raw, alpha, z, wsum
            # ============================================================
            for grp in range(NGROUP):
                pw = psacc.tile([128, H], dt.float32, name="pw", tag="psacc")
                for st in range(STAGES):
                    base = grp * GRP_SLOTS + st * SCOLS
                    bc0 = grp * BIG_PER_GROUP + st * STAGE_BIGS
                    ggs = wpool.tile([128, STAGE_BIGS, 512], dt.bfloat16, name="ggB")
                    nc.sync.dma_start(
                        ggs[:], d_Gg[bc0:bc0 + STAGE_BIGS].rearrange("c p n -> p c n"))
                    gn = wpool.tile([128, SCOLS // 128, GRP_G], dt.bfloat16,
                                    name="gnB")
                    nc.sync.dma_start(
                        gn[:],
                        d_Gn[base:base + SCOLS].rearrange("(s p) g -> p s g", p=128))
                    for bsub in range(STAGE_BIGS * 4):
                        bci, sub = divmod(bsub, 4)
                        gc = (bc0 + bci) * 4 + sub
                        s = bci * 512 + sub * 128
                        phr = psB.tile([128, H + 1], dt.float32, name="phr", tag="psB")
                        nc.tensor.matmul(phr[:, 0:H], embT0[:, gc * 128:gc * 128 + 128],
                                         W["w_rW1a"][:, 0, :], start=True, stop=False)
                        nc.tensor.matmul(phr[:, 0:H], embT1[:, gc * 128:gc * 128 + 128],
                                         W["w_rW1a"][:, 1, :], start=False, stop=False)
                        nc.tensor.matmul(phr[:, 0:H], ggs[:, bci, sub * 128:sub * 128 + 128],
                                         g_gm[:, grp, 0:H], start=False, stop=True)
                        nc.tensor.matmul(phr[:, H:H + 1],
                                         ggs[:, bci, sub * 128:sub * 128 + 128],
                                         rhi[:, grp:grp + 1], start=True, stop=False)
                        nc.tensor.matmul(phr[:, H:H + 1],
                                         ggs[:, bci, sub * 128:sub * 128 + 128],
                                         rlo[:, grp:grp + 1], start=False, stop=True)
                        nc.vector.tensor_mul(alpha_cols[:, gc:gc + 1],
                                             a_exp[:, gc:gc + 1], phr[:, H:H + 1])
                        z = wpool.tile([128, H], dt.bfloat16, name="z")
                        if USE_LRELU:
                            nc.scalar.activation(z[:], phr[:, 0:H], AF.Prelu,
                                                 scale=alpha_cols[:, gc:gc + 1],
                                                 alpha=0.1)
                        else:
                            raise NotImplementedError
                        nc.tensor.matmul(
                            pw[:], gn[:, bci * 4 + sub, :], z[:],
                            start=(st == 0 and bsub == 0),
                            stop=(st == STAGES - 1 and bsub == STAGE_BIGS * 4 - 1))
                wsum_gm = wpool.tile([128, H], dt.float32, name="wsum_gm")
                nc.scalar.activation(wsum_gm[:], pw[:], AF.Copy)
                for k in range(2):
                    ptr = psB.tile([128, 128], dt.float32, name="ptr_wsum", tag="psB")
                    nc.tensor.transpose(ptr[:], wsum_gm[:, k * 128:(k + 1) * 128],
                                        ident[:])
                    nc.vector.tensor_copy(
                        wsumT[k][:, grp * 128:(grp + 1) * 128], ptr[:])

            # ============================================================
            # Tail: vector & affinity (f32)
            # ============================================================
            for blk in range(NGROUP):
                pv = psA.tile([128, H], dt.float32, name="pv", tag="psA")
                nc.tensor.matmul(pv[:], wsumT[0][:, blk * 128:(blk + 1) * 128],
                                 W["w_rW2"][:, 0, :], start=True, stop=False)
                nc.tensor.matmul(pv[:], wsumT[1][:, blk * 128:(blk + 1) * 128],
                                 W["w_rW2"][:, 1, :], start=False, stop=False)
                nc.tensor.matmul(pv[:], sgT[0:1, blk * 128:(blk + 1) * 128],
                                 W["rawb2_row"][:], start=False, stop=True)
                vec_gm = wpool.tile([128, H], dt.float32, name="vec_gm")
                nc.scalar.activation(vec_gm[:], pv[:], AF.Copy)
                nc.sync.dma_start(d_vec[blk * 128:(blk + 1) * 128, :], vec_gm[:])
                for k in range(2):
                    ptr = psB.tile([128, 128], dt.float32, name="ptr_vec", tag="psB")
                    nc.tensor.transpose(ptr[:], vec_gm[:, k * 128:(k + 1) * 128],
                                        ident[:])
                    nc.vector.tensor_copy(vecT[k][:, blk * 128:(blk + 1) * 128], ptr[:])
            for blk in range(NGROUP):
                pt1 = psA.tile([128, H], dt.float32, name="pt1", tag="psA")
                nc.tensor.matmul(pt1[:], vecT[0][:, blk * 128:(blk + 1) * 128],
                                 W["w_oW1"][:, 0, :], start=True, stop=False)
                nc.tensor.matmul(pt1[:], vecT[1][:, blk * 128:(blk + 1) * 128],
                                 W["w_oW1"][:, 1, :], start=False, stop=False)
                nc.tensor.matmul(pt1[:], ones_f[:, 0:128], W["outb1_row"][:],
                                 start=False, stop=True)
                t1_gm = wpool.tile([128, H], dt.float32, name="t1_gm")
                leaky_dve(wpool, t1_gm[:], pt1[:])
                for k in range(2):
                    ptr = psB.tile([128, 128], dt.float32, name="ptr_t1", tag="psB")
                    nc.tensor.transpose(ptr[:], t1_gm[:, k * 128:(k + 1) * 128],
                                        ident[:])
                    nc.vector.tensor_copy(t1T[k][:, blk * 128:(blk + 1) * 128], ptr[:])
            aff_sb = ppool.tile([128, NGROUP], dt.float32, name="aff_sb")
            for blk in range(NGROUP):
                pa = psB.tile([128, 1], dt.float32, name="pa", tag="psB")
                nc.tensor.matmul(pa[:], t1T[0][:, blk * 128:(blk + 1) * 128],
                                 W["w_oW2"][:, 0, :], start=True, stop=False)
                nc.tensor.matmul(pa[:], t1T[1][:, blk * 128:(blk + 1) * 128],
                                 W["w_oW2"][:, 1, :], start=False, stop=False)
                nc.tensor.matmul(pa[:], ones_f[:, 0:128], W["outb2_row"][:],
                                 start=False, stop=True)
                nc.scalar.activation(aff_sb[:, blk:blk + 1], pa[:], AF.Copy)

            nc.sync.dma_start(d_alpha[:], alpha_cols[:])
            nc.sync.dma_start(d_aff.rearrange("(b p) one -> p (b one)", p=128),
                              aff_sb[:])
            tpool.release()
            wpool.release()
            lpool.release()

    if not nc.is_finalized():
        nc.finalize()
    return nc


_PROGRAM_CACHE = {}
TRACE = False
LAST_RESULTS = None
LAST_EXEC_WALL = None


def kernel(comp_feature, prot_feature, batch_comp, batch_prot,
           c_aff_W, c_aff_b, c_sup_W, c_sup_b, p_aff_W, p_aff_b,
           raw_W1, raw_b1, raw_W2, raw_b2,
           alpha_W1, alpha_b1, alpha_W2, alpha_b2,
           out_W1, out_b1, out_W2, out_b2):
    from concourse.bass_utils import run_bass_kernel_spmd

    inp = dict(c_aff_W=c_aff_W, c_aff_b=c_aff_b, c_sup_W=c_sup_W, c_sup_b=c_sup_b,
               p_aff_W=p_aff_W, p_aff_b=p_aff_b, raw_W1=raw_W1, raw_b1=raw_b1,
               raw_W2=raw_W2, raw_b2=raw_b2, alpha_W1=alpha_W1, alpha_b1=alpha_b1,
               alpha_W2=alpha_W2, alpha_b2=alpha_b2, out_W1=out_W1, out_b1=out_b1,
               out_W2=out_W2, out_b2=out_b2)
    inp = {k: np.asarray(v) for k, v in inp.items()}
    comp_feature = np.asarray(comp_feature, np.float32)
    prot_feature = np.asarray(prot_feature, np.float32)
    batch_comp = np.asarray(batch_comp, np.int64)
    batch_prot = np.asarray(batch_prot, np.int64)

    # dummy prot feature vector: p_aff_W^T v + b = -1e4 -> emb = -1000
    v_dummy = np.linalg.solve(np.asarray(inp["p_aff_W"], np.float64).T,
                              (-1e4 - np.asarray(inp["p_aff_b"], np.float64)))
    chk = np.asarray(inp["p_aff_W"], np.float32).T.astype(np.float32) @ \
        v_dummy.astype(BF16).astype(np.float32) + np.asarray(inp["p_aff_b"], np.float32)
    assert chk.max() < -5e3, f"v_dummy check failed: {chk.max()}"

    wmaps = _weights_maps(inp)

    in_maps = []
    slot_maps = []
    for core in range(NCORES):
        part, slot2node = _prep_core(core, comp_feature, prot_feature,
                                     batch_comp, batch_prot, v_dummy)
        part.update(wmaps)
        in_maps.append(part)
        slot_maps.append(slot2node)

    if "nc" not in _PROGRAM_CACHE:
        _PROGRAM_CACHE["nc"] = _build_program()
    nc = _PROGRAM_CACHE["nc"]

    import time as _time
    global LAST_RESULTS, LAST_EXEC_WALL
    t0 = _time.time()
    res = run_bass_kernel_spmd(nc, in_maps, core_ids=list(range(NCORES)),
                               trace=TRACE)
    LAST_EXEC_WALL = _time.time() - t0
    LAST_RESULTS = res
    results = res.results

    vector = np.zeros((B, H), np.float32)
    alpha = np.zeros((N, 1), np.float32)
    affinity = np.zeros((B, 1), np.float32)
    for core in range(NCORES):
        r = results[core]
        vector[core * GPC:(core + 1) * GPC] = r["out_vector"]
        affinity[core * GPC:(core + 1) * GPC] = r["out_aff"]
        a_cols = np.asarray(r["out_alpha"], np.float32)     # [128, 272]
        a_flat = a_cols.T.reshape(-1)                       # slot-ordered
        s2n = slot_maps[core]
        real = s2n >= 0
        alpha[s2n[real], 0] = a_flat[real]
    return vector, alpha, affinity
